# revision 55
# baseline (speedup 1.0000x reference)
"""Trainium2 Bass kernel for the BF16Indexer sparse-attention problem.

Computes, for B=1, M=2048, H=32, D=128, N=4096:
    logits = einsum('bmhd,bnd->bmhn', q, k)          (fp32 accum)
    o      = einsum('bmhn,bmh->bmn', relu(logits), w) / sqrt(D)

Sharding: M (query tokens) split across 8 cores; k replicated.

Per-core algorithm (M_loc = 256 rows, mh = M_loc*H = 8192). The n-axis is
split into an A-region (n < N_PE) and a B-region (the rest):

A-region (PE does the head-reduction, as block-diag mm2):
  - mm1 (PE): per mh-tile t (128 rows = 4 m x 32 h), psum [128, 1024]
        (2 banks) filled by 2 matmuls vs kT columns
  - drain: ONE wide relu op [128,1024] -> bf16 y (ACT mostly, some DVE)
  - mm2 (PE): block-diagonal wblk routes each tile's 4 m's to 4 of 128
        psum2 partitions, accumulated over the 32 tiles of a group

B-region (vector engines do the head-reduction; PE only does mm1):
  - mm1T (PE): stationary kT n-tile [128d,128n], stream qT -> p1T
        [128n, 1024mh] (logits transposed: partitions = n)
  - stt (DVE): z = relu(p1T) * w_rep  (w broadcast along partitions,
        softmax scale folded in; one fused scalar_tensor_tensor)
  - reduce (DVE): segmented sum over h: [128, 32m, 32h] -> [128, 32m]
        accumulated into oB[128n, 256m]
  - transpose (PE): oB -> [256m, 128n] via 2 identity matmuls, drain, DMA

Emission is a uniform token stream: 192 A-units + 64 B-chunks with B
spread over tokens [8, 242] (none in the ramp head or the tail), a
dynamic mm1->mm2 delay (8 early for wblk DMA slack, 5 in the body
so mm2 never waits on drain latency, 3 at the end), and ~1/9
of A-drains diverted from ACT to DVE. This trades ~27us of PE mm2 for
vector-engine work: PE ~207us busy, ACT ~182, DVE ~187.

Hard-won scheduling facts baked in here:
  - DMA dispatch instructions run ON the issuing engine's sequencer
    (~650ns each + queue-full waits), so ALL dma_starts go on the sync
    and gpsimd queues only - a dispatch on scalar/vector would block
    ACT/DVE compute behind it (cost ~10us of ramp when wblk/wrep sat on
    the ACT sequencer).
  - The framework startup barrier delays the first DMA dispatch to
    ~6.5us; WARMUP matmuls bridge PE activity to the first real mm1 at
    ~8us, which also keeps the HAM clock-ramp counter running (full 2.4
    GHz needs ~11us of sustained PE activity; any gap resets it and a
    mid-body down-clock costs ~10us).
  - mm2 stationaries must be full 128 cols: narrower ones trigger a
    slow PE column-group mode (+24ns/matmul); PSUM matmul outputs can
    only start at partition 0/32/64.
  - fp8 (2x PE) is numerically dead here: any single e4m3 quantization
    in the chain gives rel err ~3e-2 > the 2e-2 gate (measured).

kernel(**inputs) takes FULL inputs, returns FULL (1, 2048, 4096) fp32.
Host marshalling is layout only (transpose/replicate/scatter, no FLOPs).
Measured: ~229-231us HW exec (vs 242.8us baseline) when the device is
cool; the part thermally throttles ~20% after sustained back-to-back
runs. Rel err 3.6e-3.
"""

import math
import numpy as np
import ml_dtypes

import concourse.bass as bass
import concourse.mybir as mybir
import concourse.tile as tile
from concourse import bacc
from concourse.bass_utils import run_bass_kernel_spmd

# Problem constants (hardcoded per harness contract)
B, M, H, D, N = 1, 2048, 32, 128, 4096
N_CORES = 8
M_LOC = M // N_CORES              # 256 query rows per core
MH = M_LOC * H                    # 8192
SOFTMAX_SCALE = 1.0 / math.sqrt(float(D))

N_B = 1024                        # B-region columns (vector-engine reduction)
N_PE = N - N_B                    # A-region columns (PE block-diag reduction)
N_BT = N_B // 128                 # B-region n-tiles (8)
MH_CH = MH // 1024                # 1024-col mh chunks (8)
N_TILES = MH // 128               # 64 mh-tiles
GROUPS = [(0, 32), (32, 64)]      # A-side mm2 groups (32 tiles -> full 128-wide PE)
A_HALVES = N_PE // 1024           # 3 passes of 1024 cols per group
WARMUP = 30                       # bridge PE activity to first mm1 (~8.5us: DMA
                                  # dispatches can't start before the ~6.5us
                                  # framework startup barrier)
DELAY = 3                         # A-units of mm1->mm2 run-ahead


def _group_of(tg):
    for gi, (s, e) in enumerate(GROUPS):
        if s <= tg < e:
            return gi, s, e
    raise ValueError(tg)


def build_nc():
    nc = bacc.Bacc("TRN2", target_bir_lowering=False, debug=False)

    bf16 = mybir.dt.bfloat16
    f32 = mybir.dt.float32

    qT_d = nc.dram_tensor("qT", [128, MH], bf16, kind="ExternalInput")
    kT_d = nc.dram_tensor("kT", [128, N], bf16, kind="ExternalInput")
    wblk_d = nc.dram_tensor("wblk", [128, 128 * N_TILES], bf16, kind="ExternalInput")
    wrep_d = nc.dram_tensor("wrep", [128, MH], bf16, kind="ExternalInput")
    ident_d = nc.dram_tensor("ident", [128, 128], bf16, kind="ExternalInput")
    o_d = nc.dram_tensor("o", [M_LOC, N], f32, kind="ExternalOutput")

    relu = mybir.ActivationFunctionType.Relu

    with tile.TileContext(nc) as tc:
        with (
            tc.tile_pool(name="const", bufs=1) as const_pool,
            tc.tile_pool(name="ypool", bufs=11) as ypool,
            tc.tile_pool(name="zpool", bufs=3) as zpool,
            tc.tile_pool(name="obpool", bufs=1) as obpool,
            tc.tile_pool(name="psA", bufs=3, space="PSUM") as psA,
            tc.tile_pool(name="psum2", bufs=2, space="PSUM") as psum2,
            tc.tile_pool(name="ostage", bufs=4) as ostage,
            tc.tile_pool(name="obt", bufs=2) as obtpool,
        ):
            qT = const_pool.tile([128, MH], bf16)
            kT = const_pool.tile([128, N], bf16)
            wblk = const_pool.tile([128, 128 * N_TILES], bf16)
            wrep = const_pool.tile([128, MH], bf16)
            ident = const_pool.tile([128, 128], bf16)

            # ---- DMA prologue: need-ordered, finely chunked -------------
            def chunked(eng, dst, src, edges, width):
                lo = 0
                for hi in edges:
                    hi = min(hi, width)
                    if hi > lo:
                        eng.dma_start(dst[:, lo:hi], src[:, lo:hi])
                    lo = hi
                if lo < width:
                    eng.dma_start(dst[:, lo:], src[:, lo:])

            # sync queue (idle sequencer): kT + wblk, need-ordered.
            # scalar/vector engines get NO DMA dispatches - each dispatch
            # costs ~650ns of sequencer time and queue-full waits would
            # block ACT/DVE compute behind it.
            nc.sync.dma_start(kT[:, :512], kT_d[:, :512])
            nc.sync.dma_start(wblk[:, :256], wblk_d[:, :256])
            nc.sync.dma_start(kT[:, 512:1024], kT_d[:, 512:1024])
            nc.sync.dma_start(kT[:, N_PE:N_PE + 256], kT_d[:, N_PE:N_PE + 256])
            nc.sync.dma_start(wblk[:, 256:1024], wblk_d[:, 256:1024])
            nc.sync.dma_start(kT[:, N_PE + 256:N], kT_d[:, N_PE + 256:N])
            nc.sync.dma_start(kT[:, 1024:1536], kT_d[:, 1024:1536])
            nc.sync.dma_start(wblk[:, 1024:2048], wblk_d[:, 1024:2048])
            nc.sync.dma_start(kT[:, 1536:2048], kT_d[:, 1536:2048])
            nc.sync.dma_start(kT[:, 2048:N_PE], kT_d[:, 2048:N_PE])
            nc.sync.dma_start(wblk[:, 2048:4096], wblk_d[:, 2048:4096])
            nc.sync.dma_start(wblk[:, 4096:6144], wblk_d[:, 4096:6144])
            nc.sync.dma_start(wblk[:, 6144:], wblk_d[:, 6144:])

            # warm the ACT spline tables early
            warm = const_pool.tile([128, 1], bf16)
            nc.gpsimd.memset(warm[:], 0)
            nc.scalar.activation(warm[:], warm[:], relu)

            # warm the PE (HAM clock ramp) while DMAs land
            wz = const_pool.tile([128, 128], bf16)
            nc.gpsimd.memset(wz[:], 0)
            wps = psA.tile([128, 128], f32, tag="p1", name="warm_ps")
            for _ in range(WARMUP):
                nc.tensor.matmul(wps[:], wz[:], wz[:], start=True, stop=True)

            # gpsimd queue (idle sequencer): qT + wrep + ident.
            nc.gpsimd.dma_start(qT[:, :256], qT_d[:, :256])
            nc.gpsimd.dma_start(qT[:, 256:512], qT_d[:, 256:512])
            nc.gpsimd.dma_start(qT[:, 512:1024], qT_d[:, 512:1024])
            nc.gpsimd.dma_start(wrep[:, :512], wrep_d[:, :512])
            nc.gpsimd.dma_start(wrep[:, 512:1024], wrep_d[:, 512:1024])
            nc.gpsimd.dma_start(qT[:, 1024:1536], qT_d[:, 1024:1536])
            nc.gpsimd.dma_start(qT[:, 1536:2048], qT_d[:, 1536:2048])
            nc.gpsimd.dma_start(wrep[:, 1024:2048], wrep_d[:, 1024:2048])
            nc.gpsimd.dma_start(qT[:, 2048:3072], qT_d[:, 2048:3072])
            nc.gpsimd.dma_start(qT[:, 3072:4096], qT_d[:, 3072:4096])
            nc.gpsimd.dma_start(wrep[:, 2048:4096], wrep_d[:, 2048:4096])
            nc.gpsimd.dma_start(qT[:, 4096:6144], qT_d[:, 4096:6144])
            nc.gpsimd.dma_start(qT[:, 6144:], qT_d[:, 6144:])
            nc.gpsimd.dma_start(wrep[:, 4096:6144], wrep_d[:, 4096:6144])
            nc.gpsimd.dma_start(wrep[:, 6144:], wrep_d[:, 6144:])
            nc.gpsimd.dma_start(ident[:], ident_d[:])

            # ---- A-side emitters ---------------------------------------
            drain_ctr = [0]

            def emit_A_mm1(gi, hf, t, use_dve_drain=False):
                tg = GROUPS[gi][0] + t
                qT_t = qT[:, bass.ts(tg, 128)]
                p = psA.tile([128, 1024], f32, tag="p1", name=f"pA_{gi}_{hf}_{t}")
                for c in range(2):
                    nc.tensor.matmul(
                        p[:, bass.ts(c, 512)], qT_t,
                        kT[:, bass.ds(hf * 1024 + c * 512, 512)],
                        start=True, stop=True)
                y_t = ypool.tile([128, 1024], bf16, tag="y", name=f"y_{gi}_{hf}_{t}")
                drain_ctr[0] += 1
                if use_dve_drain:
                    nc.vector.tensor_scalar(
                        y_t[:], p[:], SOFTMAX_SCALE, 0.0,
                        mybir.AluOpType.mult, mybir.AluOpType.max)
                else:
                    nc.scalar.activation(y_t[:], p[:], relu, scale=SOFTMAX_SCALE)
                return y_t

            def emit_A_mm2(p2_chunks, gi, t, y_t):
                # full-width block-diagonal stationary: tile's 4 m's land in
                # cols 4t..4t+3 of the group's 128 psum2 partitions
                s, e = GROUPS[gi]
                tg = s + t
                w_t = wblk[:, bass.ts(tg, 128)]
                for c in range(2):
                    nc.tensor.matmul(
                        p2_chunks[c][:], w_t, y_t[:, bass.ts(c, 512)],
                        start=(t == 0), stop=(t == e - s - 1))

            def finish_A_pass(gi, hf, p2_chunks, last):
                s, e = GROUPS[gi]
                gp = 4 * (e - s)
                for c in range(2):
                    ost = ostage.tile([gp, 512], f32, tag="ost",
                                      name=f"ost_{gi}_{hf}_{c}")
                    if last and c == 0:
                        nc.scalar.copy(ost[:], p2_chunks[c][:])
                    else:
                        nc.vector.tensor_copy(ost[:], p2_chunks[c][:])
                    base = hf * 1024 + c * 512
                    if last:
                        # gpsimd dispatched first and lighter (32 vs 96
                        # rows): its end-of-kernel DGE flush is on the
                        # critical path while sync's queue drains early
                        nc.gpsimd.dma_start(
                            o_d[bass.ds(4 * s + 96, 32), bass.ds(base, 512)],
                            ost[96:128, :])
                        nc.sync.dma_start(
                            o_d[bass.ds(4 * s, 96), bass.ds(base, 512)],
                            ost[0:96, :])
                    else:
                        (nc.sync if c == 0 else nc.gpsimd).dma_start(
                            o_d[bass.ds(4 * s, gp), bass.ds(base, 512)], ost[:])

            # ---- B-side emitters ---------------------------------------
            oB = [obpool.tile([128, M_LOC], bf16, tag=f"oB{j}", name=f"oB_{j}")
                  for j in range(N_BT)]

            def emit_B_chunk(j, c):
                kT_j = kT[:, bass.ds(N_PE + j * 128, 128)]
                pb = psA.tile([128, 1024], f32, tag="p1", name=f"pB_{j}_{c}")
                for s in range(2):
                    nc.tensor.matmul(
                        pb[:, bass.ts(s, 512)], kT_j,
                        qT[:, bass.ds(c * 1024 + s * 512, 512)],
                        start=True, stop=True)
                z = zpool.tile([128, 1024], bf16, tag="z", name=f"z_{j}_{c}")
                nc.vector.scalar_tensor_tensor(
                    z[:], pb[:], 0.0, wrep[:, bass.ts(c, 1024)],
                    op0=mybir.AluOpType.max, op1=mybir.AluOpType.mult)
                with nc.allow_low_precision("bf16 oB keeps PE transposes fast"):
                    nc.vector.tensor_reduce(
                        oB[j][:, bass.ts(c, 32)],
                        z[:].rearrange("p (m h) -> p m h", h=32),
                        axis=mybir.AxisListType.X, op=mybir.AluOpType.add)

            def emit_B_finish(j):
                tp = psA.tile([128, 256], bf16, tag="p1", name=f"tp_{j}")
                nc.tensor.transpose(tp[:, :128], oB[j][:, :128], ident[:])
                nc.tensor.transpose(tp[:, 128:], oB[j][:, 128:], ident[:])
                obt = obtpool.tile([128, 256], f32, tag="obt", name=f"obt_{j}")
                nc.vector.tensor_copy(obt[:], tp[:])
                cols = bass.ds(N_PE + j * 128, 128)
                nc.gpsimd.dma_start(o_d[0:128, cols], obt[:, :128])
                nc.sync.dma_start(o_d[128:256, cols], obt[:, 128:])

            # ---- interleaved emission ----------------------------------
            a_units = [(gi, hf, t)
                       for gi, (s, e) in enumerate(GROUPS)
                       for hf in range(A_HALVES)
                       for t in range(e - s)]
            b_chunks = [(j, c) for c in range(MH_CH) for j in range(N_BT)]
            assert len(a_units) == 3 * len(b_chunks)

            p2_of = {}
            ys = {}
            a_idx = 0
            emitted_mm2 = 0

            def pump_A_mm2(upto):
                nonlocal emitted_mm2
                while emitted_mm2 < upto:
                    ju = emitted_mm2
                    gi, hf, t = a_units[ju]
                    s, e = GROUPS[gi]
                    pi = (gi, hf)
                    if pi not in p2_of:
                        p2_of[pi] = [
                            psum2.tile([4 * (e - s), 512], f32, tag="p2",
                                       name=f"p2_{gi}_{hf}_{c}")
                            for c in range(2)]
                    emit_A_mm2(p2_of[pi], gi, t, ys.pop(ju))
                    if t == e - s - 1:
                        last = (gi == len(GROUPS) - 1 and hf == A_HALVES - 1)
                        finish_A_pass(gi, hf, p2_of.pop(pi), last)
                    emitted_mm2 += 1

            # Token stream: 192 A-units + 64 B-chunks. B is spread over
            # tokens [8, 242]: none in the head (ramp: pure mm1 on qT/kT
            # only, mm2 delayed 8 units for wblk slack) and none in the
            # tail (the last ~13 A-units hide the final B chain).
            b_pos = {8 + round(k * 234 / 63): k for k in range(len(b_chunks))}
            assert len(b_pos) == len(b_chunks)
            n_tokens = len(a_units) + len(b_chunks)
            tok_of = {}
            ai = 0
            for tok in range(n_tokens):
                if tok not in b_pos:
                    tok_of[tok] = ("A", ai)
                    ai += 1
            assert ai == len(a_units)

            for tok in range(n_tokens):
                if tok in b_pos:
                    j, c = b_chunks[b_pos[tok]]
                    emit_B_chunk(j, c)
                    if c == MH_CH - 1:
                        emit_B_finish(j)
                else:
                    _, ui = tok_of[tok]
                    gi, hf, t = a_units[ui]
                    dve_drain = (ui % 9 == 4)
                    ys[ui] = emit_A_mm1(gi, hf, t, use_dve_drain=dve_drain)
                    a_idx += 1
                    # mm1->mm2 lag: 8 units early (wblk DMA slack), 5 in the
                    # body (mm2 never waits on ACT drain latency), 3 at the
                    # end (short trailing flush)
                    if a_idx <= 30:
                        delay = 8
                    elif a_idx <= len(a_units) - 24:
                        delay = 5
                    else:
                        delay = DELAY
                    pump_A_mm2(max(0, a_idx - delay))
            pump_A_mm2(len(a_units))

    nc.compile()
    return nc


def marshal_core_inputs(q, k, weights, core):
    """Host-side layout marshalling for one core (no arithmetic)."""
    bf16 = ml_dtypes.bfloat16
    f32 = np.float32

    q_sh = np.asarray(q[0, core * M_LOC:(core + 1) * M_LOC])   # (m_loc, H, D)
    qT = np.ascontiguousarray(q_sh.reshape(MH, D).T)            # (128, mh)
    kT = np.ascontiguousarray(np.asarray(k[0]).T)               # (128, n)

    w_sh = np.asarray(weights[core * M_LOC:(core + 1) * M_LOC, 0, :])  # (m_loc, H)

    # A-side block-diagonal mm2 stationaries: tile tg's [128, 128] block
    # maps its 4 m's to cols 4*(tg - group_start) + jj
    # (no scale; ACT drain applies it)
    w_r = w_sh.reshape(N_TILES, 4, H)                           # (tg, jj, h)
    gstart = np.zeros(N_TILES, dtype=np.int64)
    for (s, e) in GROUPS:
        gstart[s:e] = s
    tgs = np.arange(N_TILES)
    wblk = np.zeros((N_TILES, 128, 128), dtype=bf16)
    for jj in range(4):
        cols = 4 * (tgs - gstart) + jj
        wblk[tgs[:, None], 32 * jj + np.arange(H)[None, :], cols[:, None]] = \
            w_r[:, jj, :]
    wblk = np.ascontiguousarray(
        wblk.transpose(1, 0, 2).reshape(128, 128 * N_TILES))

    # B-side replicated row weights WITH softmax scale folded in
    wrow = (w_sh.astype(f32) * np.float32(SOFTMAX_SCALE)).astype(bf16).reshape(1, MH)
    wrep = np.ascontiguousarray(np.broadcast_to(wrow, (128, MH)))

    ident = np.eye(128, dtype=bf16)

    return {"qT": qT, "kT": kT, "wblk": wblk, "wrep": wrep, "ident": ident}


_NC_CACHE = {}


def _get_nc():
    if "nc" not in _NC_CACHE:
        _NC_CACHE["nc"] = build_nc()
    return _NC_CACHE["nc"]


def kernel(q, k, weights):
    nc = _get_nc()
    in_maps = [marshal_core_inputs(q, k, weights, c) for c in range(N_CORES)]
    res = run_bass_kernel_spmd(nc, in_maps, list(range(N_CORES)))
    out = np.concatenate([res.results[c]["o"] for c in range(N_CORES)], axis=0)
    return out[None]  # (1, M, N) fp32
